# revision 2
# baseline (speedup 1.0000x reference)
"""Trainium2 Bass kernel for the KAN autonomous ODE func:
    s   = tanh(h[:, :, None] * alpha + beta)            # [B, H, K]
    phi = einsum("bik,oik->bo", s, W) / K               # [B, O]
    out = tanh(phi) * gain + bias                       # [B, O]
with B=2048, H=1024, K=16, O=H.

v3: mixed-precision slab compression. The K=16 tanh bases are fit by
{1, x} + 6 nonlinear units; the change of basis folds into W
(W2[o,i,m]). Unit matmuls run fp8-e4m3 DoubleRow (2 i-chunks per MM;
HW-measured 217 ns/MM at N=512 = 2x bf16 FLOPs); the x slab stays bf16.
fp8 noise control:
  - tanh units are CENTERED on chip (slab = tanh(ah+b) - c1*h - c0,
    linear part folded into the x/const columns host-side), shrinking
    slab variance ~10-20x -> fp8 noise of both the slab and its weights
    scales down by the same factor.
  - sin units are constrained to a >= 1.5 where the linear projection
    is already ~0 (E[sin'] ~ a e^{-a^2/2}), so the ACT engine emits
    their fp8 slabs directly with no centering ops.
  - a square unit (h^2 - 1) is built on the vector engine.
  - W-side fp8 error is GPTQ-compensated into not-yet-quantized columns.
Engine budget per core: PE 32 bf16 + 96 DR MMs ~ 28 us; scalar 5 ACT
slabs + epilogue ~ 26 us; DVE square + 3 subtracts ~ 20 us; gpsimd
3 v-slabs + DMA triggers ~ 20 us.

Sharding (8 cores): 4 batch shards x 2 output shards, no collectives.
"""

import sys

import numpy as np

if "/opt/trn_rl_repo" not in sys.path:
    sys.path.insert(0, "/opt/trn_rl_repo")

import ml_dtypes

import concourse.bass as bass
import concourse.tile as tile
from concourse import bacc, mybir
from concourse.bass_utils import run_bass_kernel_spmd

B, H, K = 2048, 1024, 16
RB, CO = 4, 2
B_SH = B // RB
O_SH = H // CO
NCH = 8                       # i-chunks of 128
NQ = 4                        # quarters (2 chunks each)
QCH = NCH // NQ
OT = O_SH // 128
LAM = 256.0                   # global weight scale (power of 2)

F32 = mybir.dt.float32
BF16 = mybir.dt.bfloat16
FP8 = mybir.dt.float8e4

AF = mybir.ActivationFunctionType
ALU = mybir.AluOpType
DRPM = mybir.MatmulPerfMode.DoubleRow

FUNC_ENUM = {"tanh": AF.Tanh, "sin": AF.Sin, "silu": AF.Silu}

_CACHE = {}

bf = lambda x: np.asarray(x, dtype=ml_dtypes.bfloat16).astype(np.float32)


def fp8q(x):
    y = np.clip(np.asarray(x, np.float32), -240.0, 240.0)
    return np.asarray(y, dtype=ml_dtypes.float8_e4m3).astype(np.float32)


# ---------------------------------------------------------------------------
# Host-side fit (numpy only, deterministic)
# ---------------------------------------------------------------------------

def _np_funcs(t, z):
    if t == "tanh":
        return np.tanh(z)
    if t == "sin":
        return np.sin(z)
    if t == "silu":
        return z / (1.0 + np.exp(-np.clip(z, -60, 60)))
    if t == "square":
        return z * z
    raise KeyError(t)


def _np_dfuncs(t, z):
    if t == "tanh":
        c = np.cosh(np.clip(z, -30, 30))
        return 1.0 / (c * c)
    if t == "sin":
        return np.cos(z)
    if t == "silu":
        ez = np.exp(-np.clip(z, -60, 60))
        return (1.0 + ez * (1.0 + z)) / (1.0 + ez) ** 2
    if t == "square":
        return 2.0 * z
    raise KeyError(t)


XG = np.linspace(-5.6, 5.6, 4481)
WG = np.exp(-0.5 * XG * XG)
WG /= WG.sum()
SWG = np.sqrt(WG)


def _fit_units_seq(alpha, beta, type_seq, amin_map, ridge=6e-6,
                   fixed_units=()):
    """Greedy (fixed per-step unit type) + variable-projection GN refinement
    with per-type lower bounds on the frequency a."""
    T = np.tanh(np.outer(alpha, XG) + beta[:, None])
    Yw = (T * SWG).T
    fixed_t = [u[0] for u in fixed_units]
    fixed_p = [(float(u[1]), float(u[2])) for u in fixed_units]

    def design(free_params, free_types):
        rows = [np.ones_like(XG), XG]
        for t, (a, b) in zip(fixed_t + list(free_types),
                             fixed_p + list(free_params)):
            rows.append(_np_funcs(t, a * XG + b))
        return np.stack(rows)

    def solve(free_params, free_types):
        Phi = design(free_params, free_types)
        A = (Phi * SWG).T
        colnorm = np.sqrt((Phi**2 * WG).sum(axis=1))
        colnorm[0] = 0.0
        D = np.sqrt(ridge) * np.diag(colnorm)
        A_aug = np.vstack([A, D])
        Y_aug = np.vstack([Yw, np.zeros((A.shape[1], Yw.shape[1]))])
        C, *_ = np.linalg.lstsq(A_aug, Y_aug, rcond=None)
        return C, A_aug, Y_aug

    b_pool = np.linspace(-3.5, 3.5, 57)
    types, params = [], []
    for step_t in type_seq:
        amin = amin_map.get(step_t, 0.1)
        a_pool = np.linspace(max(amin, 0.1), 6.0, 71)
        AA, BB = np.meshgrid(a_pool, b_pool)
        P = np.stack([AA.ravel(), BB.ravel()], axis=1)
        V = _np_funcs(step_t, P[:, 0:1] * XG[None, :] + P[:, 1:2]) * SWG
        A = (design(params, types) * SWG).T
        Q, _ = np.linalg.qr(A)
        Rm = Yw.T - (Yw.T @ Q) @ Q.T
        Vp = V - (V @ Q) @ Q.T
        nrm = np.linalg.norm(Vp, axis=1) + 1e-12
        sc = np.linalg.norm(Rm @ Vp.T / nrm, axis=0)
        i = int(np.argmax(sc))
        types.append(step_t)
        params.append((float(P[i][0]), float(P[i][1])))

    NBASE = 2

    def residual_and_jac(free_params):
        C, A_aug, Y_aug = solve(free_params, types)
        R = Y_aug - A_aug @ C
        Q, _ = np.linalg.qr(A_aug)
        cols = []
        G = len(XG)
        off = NBASE + len(fixed_units)
        for j, (t, (a, b)) in enumerate(zip(types, free_params)):
            z = a * XG + b
            d = _np_dfuncs(t, z)
            for which in (0, 1):
                dcol = (d * (XG if which == 0 else 1.0)) * SWG
                dA = np.zeros((A_aug.shape[0], Yw.shape[1]))
                dA[:G] = dcol[:, None] * C[off + j][None, :]
                dA -= Q @ (Q.T @ dA)
                cols.append(-dA.ravel())
        J = np.stack(cols, axis=1)
        return R.ravel(), J

    p = np.array(params, np.float64)
    amins = np.array([amin_map.get(t, 0.1) for t in types])
    lam = 1e-3
    r0, _ = residual_and_jac(params)
    f0 = float(r0 @ r0)
    for _ in range(60):
        r, Jm = residual_and_jac([tuple(q) for q in p])
        g = Jm.T @ r
        Hm = Jm.T @ Jm
        step = np.linalg.solve(Hm + lam * np.diag(np.diag(Hm) + 1e-12), -g)
        p_new = p + step.reshape(-1, 2)
        p_new[:, 0] = np.clip(p_new[:, 0], amins, 8.0)
        r_new, _ = residual_and_jac([tuple(q) for q in p_new])
        f_new = float(r_new @ r_new)
        if f_new < f0:
            p, f0, lam = p_new, f_new, max(lam * 0.3, 1e-8)
        else:
            lam = min(lam * 4.0, 1e4)
    params = [tuple(q) for q in p]
    return fixed_t + types, fixed_p + params


def _cols_mixed(types, params, center_mask):
    """Design columns: centered where mask (and for square), raw otherwise.
    Returns (U [nu, G], lin [(c0, c1)])."""
    A = np.stack([np.ones_like(XG), XG])
    G2 = (A * WG) @ A.T
    U, lin = [], []
    for (t, (a, b)), cen in zip(zip(types, params), center_mask):
        u = _np_funcs(t, a * XG + b)
        if t == "square":
            U.append(u - 1.0)
            lin.append((1.0, 0.0))
        elif cen:
            c = np.linalg.solve(G2, (A * WG) @ u)
            U.append(u - c[0] - c[1] * XG)
            lin.append((float(c[0]), float(c[1])))
        else:
            U.append(u)
            lin.append((0.0, 0.0))
    return np.stack(U), lin


def _refit(types, params, alpha, beta, U, lam=1e-3):
    """LS fit in basis {1, x, U} with colnorm-scaled ridge on unit cols."""
    T = np.tanh(np.outer(alpha, XG) + beta[:, None])
    Phi = np.vstack([np.ones_like(XG), XG, U])
    A = (Phi * SWG).T
    Y = (T * SWG).T
    colnorm = np.sqrt((Phi**2 * WG).sum(axis=1))
    pen = np.zeros(len(colnorm))
    pen[2:] = np.sqrt(lam) * colnorm[2:]
    A_aug = np.vstack([A, np.diag(pen)])
    Y_aug = np.vstack([Y, np.zeros((len(pen), K))])
    C, *_ = np.linalg.lstsq(A_aug, Y_aug, rcond=None)
    return C.T                                           # [K, 2+nu]


def _quantize_weights(W, C, U):
    """Fold basis into W; GPTQ-quantize unit cols to fp8(xLAM), x to bf16."""
    nu = U.shape[0]
    C64 = (C / K).astype(np.float64)
    W2full = (W.reshape(H * H, K).astype(np.float64) @ C64).reshape(H, H, 2 + nu)
    phi_bias = W2full[:, :, 0].sum(axis=1).astype(np.float32)
    W2 = W2full[:, :, 1:].copy()                         # [o, i, 1+nu]
    Phi = np.vstack([XG, U])
    G = (Phi * WG) @ Phi.T
    M = 1 + nu
    Wq = W2.copy()
    for m in range(1, M):
        q = (fp8q(Wq[:, :, m] * LAM) / LAM).astype(np.float64)
        eps = Wq[:, :, m] - q
        Wq[:, :, m] = q
        rem = [r for r in range(M) if r == 0 or r > m]
        cvec = np.linalg.solve(G[np.ix_(rem, rem)], G[rem, m])
        for j, r in enumerate(rem):
            Wq[:, :, r] += eps * cvec[j]
    wx = np.asarray(Wq[:, :, 0] * LAM, np.float32).astype(ml_dtypes.bfloat16)
    wu = np.clip(np.transpose(Wq[:, :, 1:], (2, 0, 1)) * LAM, -240.0, 240.0)
    wu = np.asarray(wu, np.float32).astype(ml_dtypes.float8_e4m3)
    return wx, wu, phi_bias


def _sim_err(h_sub, W, alpha, beta, types, params, lin, center_mask,
             wx, wu, phi_bias, gain, bias):
    """Device-numerics simulation vs fp32 reference on a batch subsample."""
    nsub = len(h_sub)
    s_ref = np.tanh(h_sub[:, :, None] * alpha.astype(np.float32)
                    + beta.astype(np.float32))
    phi_ref = s_ref.reshape(nsub, H * K) @ W.reshape(H, H * K).T / K
    ref = np.tanh(phi_ref) * gain + bias
    hq = bf(h_sub)
    psum = hq @ np.asarray(wx, np.float32).T
    for m, ((t, (a, b)), (c0, c1), cen) in enumerate(
            zip(zip(types, params), lin, center_mask)):
        if t == "square":
            su = fp8q(bf(hq * hq) - 1.0)
        elif cen:
            s_tmp = bf(_np_funcs(t, np.float32(a) * hq + np.float32(b)))
            v = bf(np.float32(c1) * hq + np.float32(c0))
            su = fp8q(s_tmp - v)
        else:
            su = fp8q(_np_funcs(t, np.float32(a) * hq + np.float32(b)))
        psum += su @ np.asarray(wu[m], np.float32).T
    outv = np.tanh(psum * np.float32(1.0 / LAM) + phi_bias)
    out = bf(outv * gain + bias)
    return float(np.linalg.norm(out - ref) / np.linalg.norm(ref))


def _prep(h, W, alpha, beta, gain, bias):
    """Select config, fit, quantize, pack per-core inputs."""
    h = np.asarray(h, np.float32)
    W = np.asarray(W, np.float32)
    alpha = np.asarray(alpha, np.float64)
    beta = np.asarray(beta, np.float64)
    gain = np.asarray(gain, np.float32)
    bias = np.asarray(bias, np.float32)
    h_sub = np.ascontiguousarray(h[:512])

    sq = (("square", 1.0, 0.0),)
    cands = [
        (["tanh", "tanh", "tanh", "tanh"], {}),
        (["tanh", "tanh", "tanh", "tanh", "tanh"], {}),
    ]
    best = None
    chosen = None
    for seq, amin_map in cands:
        types, params = _fit_units_seq(alpha, beta, seq, amin_map,
                                       fixed_units=sq)
        cen = [t == "tanh" for t in types]
        U, lin = _cols_mixed(types, params, cen)
        C = _refit(types, params, alpha, beta, U, lam=1e-3)
        wx, wu, phi_bias = _quantize_weights(W, C, U)
        err = _sim_err(h_sub, W, alpha, beta, types, params, lin, cen,
                       wx, wu, phi_bias, gain, bias)
        pack = (types, params, lin, cen, wx, wu, phi_bias, err)
        if best is None or err < best[7]:
            best = pack
        if err <= 1.45e-2:
            chosen = pack
            break
    if chosen is None:
        chosen = best
    types, params, lin, cen, wx, wu, phi_bias, err = chosen

    # device unit descriptors in PE order (= fit order)
    units = []
    scalar_i = 0
    cc_i = 0
    for t, c in zip(types, cen):
        if t == "square":
            units.append(("sq", -1, -1))
        elif c:
            units.append(("ct", scalar_i, cc_i))
            scalar_i += 1
            cc_i += 1
        else:
            units.append((t[:2], scalar_i, -1))          # 'ta'/'si' raw
            scalar_i += 1
    nsu, ncc = scalar_i, cc_i
    ukinds = tuple(u[0] for u in units)
    stypes = tuple(t for t in types if t != "square")

    def to_pc(A2):  # [i, X] -> [128, NCH, X]
        return np.ascontiguousarray(
            A2.reshape(NCH, 128, A2.shape[1]).transpose(1, 0, 2))

    wxT = to_pc(np.ascontiguousarray(np.asarray(wx, np.float32).T)).astype(
        ml_dtypes.bfloat16)
    wuT = np.stack([
        to_pc(np.ascontiguousarray(np.asarray(wu[m], np.float32).T)).astype(
            ml_dtypes.float8_e4m3)
        for m in range(len(units))
    ])

    sidx = [i for i, t in enumerate(types) if t != "square"]
    a_arr = np.array([params[i][0] for i in sidx], np.float32)
    b_arr = np.array([params[i][1] for i in sidx], np.float32)
    ab = np.ascontiguousarray(
        np.tile(np.concatenate([a_arr, b_arr])[None, :], (128, 1))).astype(
            np.float32)
    cidx = [i for i, (t, c) in enumerate(zip(types, cen)) if c and t != "square"]
    c1_arr = np.array([lin[i][1] for i in cidx], np.float32)
    c0_arr = np.array([lin[i][0] for i in cidx], np.float32)
    if ncc == 0:
        ccm = np.zeros((128, 2), np.float32)
    else:
        ccm = np.ascontiguousarray(
            np.tile(np.concatenate([c1_arr, c0_arr])[None, :], (128, 1))
        ).astype(np.float32)

    in_maps = []
    for rb in range(RB):
        h_sh = h[rb * B_SH: (rb + 1) * B_SH, :]
        hTv = h_sh.T.reshape(NCH, 128, B_SH).transpose(1, 0, 2)
        hT = np.ascontiguousarray(hTv).astype(ml_dtypes.bfloat16)
        for co in range(CO):
            osl = slice(co * O_SH, (co + 1) * O_SH)
            g = gain[osl].reshape(OT, 128).T
            bb = bias[osl].reshape(OT, 128).T
            pb = phi_bias[osl].reshape(OT, 128).T
            gbv = np.ascontiguousarray(
                np.concatenate([g, bb, pb], axis=1)).astype(np.float32)
            in_maps.append({
                "hT": hT,
                "wx": np.ascontiguousarray(wxT[:, :, osl]),
                "wu": np.ascontiguousarray(wuT[:, :, :, osl]),
                "ab": ab,
                "cc": ccm,
                "gb": gbv,
            })
    return (ukinds, stypes), in_maps, err


# ---------------------------------------------------------------------------
# Device kernel
# ---------------------------------------------------------------------------

def _patch_act_tables():
    """Restrict the act-table chooser to silu_and_others (has tanh+sin+silu)
    so mixed tanh/sin kernels need exactly one ACT_TABLE_LOAD. Other sets are
    emptied (not removed) to preserve act_func_set_id indices."""
    import concourse.bacc as _bacc
    import concourse.hw_specs as _hw
    if getattr(_bacc, "_act_tables_patched", False):
        return
    real = _hw.get_activation_tables

    def patched(module_arch):
        tabs = real(module_arch)
        return {name: (fns if name == "silu_and_others" else set())
                for name, fns in tabs.items()}

    import os
    if os.environ.get("K2_PATCH_ACT", "1") == "1":
        _bacc.get_activation_tables = patched
    _bacc._act_tables_patched = True


def _build(ukinds, stypes, fuse_gain_bias):
    key = ("v5", ukinds, stypes, fuse_gain_bias)
    if key in _CACHE:
        return _CACHE[key]
    nuf = len(ukinds)
    nsu = len(stypes)
    ncc = sum(1 for k in ukinds if k == "ct")
    NH = 2                    # halves for slab production
    HCH = NCH // NH

    nc = bacc.Bacc(
        "TRN2",
        target_bir_lowering=False,
        debug=False,
        enable_asserts=False,
        num_devices=RB * CO,
    )

    hT = nc.dram_tensor("hT", [128, NCH, B_SH], BF16, kind="ExternalInput").ap()
    wx = nc.dram_tensor("wx", [128, NCH, O_SH], BF16, kind="ExternalInput").ap()
    wu = nc.dram_tensor("wu", [nuf, 128, NCH, O_SH], FP8,
                        kind="ExternalInput").ap()
    ab = nc.dram_tensor("ab", [128, 2 * nsu], F32, kind="ExternalInput").ap()
    cc = nc.dram_tensor("cc", [128, max(2 * ncc, 2)], F32,
                        kind="ExternalInput").ap()
    gb = nc.dram_tensor("gb", [128, 3 * OT], F32, kind="ExternalInput").ap()
    out = nc.dram_tensor("out", [OT, 128, B_SH], BF16, kind="ExternalOutput").ap()

    # unit descriptors in fit order
    UNITS = []
    scalar_i = 0
    cc_i = 0
    for kind in ukinds:
        if kind == "sq":
            UNITS.append(("sq", -1, -1))
        elif kind == "ct":
            UNITS.append(("ct", scalar_i, cc_i))
            scalar_i += 1
            cc_i += 1
        else:
            UNITS.append((kind, scalar_i, -1))
            scalar_i += 1
    ct_list = [j for j in range(nuf) if UNITS[j][0] == "ct"]
    pe_order = ct_list + [j for j in range(nuf) if UNITS[j][0] != "ct"]

    with tile.TileContext(nc) as tc:
        with (
            tc.tile_pool(name="const", bufs=1) as const_pool,
            tc.tile_pool(name="h", bufs=1) as h_pool,
            tc.tile_pool(name="wx0", bufs=1) as wx_pool,
            tc.tile_pool(name="wu", bufs=1) as wu_pool,
            tc.tile_pool(name="stmp", bufs=3) as stmp_pool,
            tc.tile_pool(name="v", bufs=3) as v_pool,
            tc.tile_pool(name="su", bufs=3) as su_pool,
            tc.tile_pool(name="o", bufs=2) as o_pool,
            tc.tile_pool(name="psum", bufs=1, space=bass.MemorySpace.PSUM) as psum_pool,
        ):
            # ---- gpsimd: consts, odd wx chunks, one unit W ----
            ab_t = const_pool.tile([128, 2 * nsu], F32, tag="ab")
            nc.gpsimd.dma_start(ab_t[:], ab[:])
            cc_t = const_pool.tile([128, max(2 * ncc, 2)], F32, tag="cc")
            nc.gpsimd.dma_start(cc_t[:], cc[:])
            gb_t = const_pool.tile([128, 3 * OT], F32, tag="gb")
            nc.gpsimd.dma_start(gb_t[:], gb[:])

            wx_c = [
                wx_pool.tile([128, 1, O_SH], BF16, tag=f"wx{c}", name=f"wx{c}")
                for c in range(NCH)
            ]
            for c in range(0, NCH, 2):
                nc.sync.dma_start(wx_c[c][:], wx[:, c: c + 1, :])
            for c in range(1, NCH, 2):
                nc.gpsimd.dma_start(wx_c[c][:], wx[:, c: c + 1, :])

            # ---- scalar: h quarters only up front ----
            h_q = [
                h_pool.tile([128, QCH, B_SH], BF16, tag=f"h{q}", name=f"h_q{q}")
                for q in range(NQ)
            ]
            for q in range(NQ):
                nc.scalar.dma_start(h_q[q][:], hT[:, q * QCH: (q + 1) * QCH, :])

            def h_half(hh):
                # [128, HCH, B_SH] view over two h quarter tiles is not
                # possible; produce per-half APs from quarter tiles pairwise
                return (h_q[2 * hh], h_q[2 * hh + 1])

            # ---- unit W tiles ----
            wu_q = {}

            def wu_tile(m, q):
                if (m, q) not in wu_q:
                    wu_q[(m, q)] = wu_pool.tile(
                        [128, QCH, O_SH], FP8, tag=f"wu{m}_{q}",
                        name=f"wu_{m}_{q}", bufs=1)
                return wu_q[(m, q)]

            wu_whole = {}

            def wu_whole_tile(m):
                if m not in wu_whole:
                    wu_whole[m] = wu_pool.tile(
                        [128, NCH, O_SH], FP8, tag=f"wuW{m}",
                        name=f"wu_whole_{m}", bufs=1)
                return wu_whole[m]

            def dma_wu_quarters(eng, m):
                for q in range(NQ):
                    eng.dma_start(wu_tile(m, q)[:],
                                  wu[m, :, q * QCH: (q + 1) * QCH, :])

            def wu_ap(m, q):
                if m in wu_whole:
                    return wu_whole[m][:, q * QCH: (q + 1) * QCH, :]
                return wu_tile(m, q)[:]

            # W streams, PE-consumption order: ct units on sync/gpsimd/sync;
            # remaining ct + tail (sq) via scalar mid-stream whole-slabs
            sync_units = pe_order[0:3:2]           # ct1, ct3
            gp_units = pe_order[1:2]               # ct2
            scalar_mid_units = pe_order[3:]        # ct4, sq
            for m in sync_units:
                dma_wu_quarters(nc.sync, m)
            for m in gp_units:
                dma_wu_quarters(nc.gpsimd, m)
            for m in scalar_mid_units:
                wu_whole_tile(m)

            psum_b = [
                psum_pool.tile([128, B_SH], F32, tag=f"acc{ot}", name=f"acc{ot}")
                for ot in range(OT)
            ]

            # ---- HAM warm-up ----
            warm_sb = const_pool.tile([128, 128], BF16, tag="warm")
            nc.vector.memset(warm_sb[:], 0.0)
            warm_ps = psum_pool.tile([128, 128], F32, tag="warmps")
            N_WARM = 48
            for i in range(N_WARM):
                nc.tensor.matmul(warm_ps[:], warm_sb[:], warm_sb[:],
                                 start=(i == 0), stop=(i == N_WARM - 1))

            # ---- x slab ----
            for c in range(NCH):
                for ot in range(OT):
                    nc.tensor.matmul(
                        psum_b[ot][:],
                        wx_c[c][:, 0, ot * 128: (ot + 1) * 128],
                        h_q[c // QCH][:, c % QCH, :],
                        start=(c == 0),
                        stop=False,
                    )

            # ---- slab production at half granularity ----
            # per half hh: su_half tile [128, HCH=4, B_SH]; MMs consume
            # quarter slices su[:, 0:2 / 2:4, :]
            def make_su_half(j, hh):
                kind, si, ci = UNITS[j]
                q0, q1 = 2 * hh, 2 * hh + 1
                if kind == "sq":
                    su_t = su_pool.tile([128, HCH, B_SH], FP8, tag=f"susq{hh}",
                                        name=f"su_{j}_{hh}", bufs=1)
                    st = stmp_pool.tile([128, HCH, B_SH], BF16, tag=f"stsq{hh}",
                                        bufs=1)
                    nc.vector.tensor_mul(st[:, 0:QCH, :], h_q[q0][:], h_q[q0][:])
                    nc.vector.tensor_mul(st[:, QCH:HCH, :], h_q[q1][:], h_q[q1][:])
                    nc.vector.tensor_scalar_sub(su_t[:], st[:], 1.0)
                    return su_t
                su_t = su_pool.tile([128, HCH, B_SH], FP8, tag=f"su{hh}",
                                    name=f"su_{j}_{hh}")
                st = stmp_pool.tile([128, HCH, B_SH], BF16, tag=f"st{hh}")
                nc.scalar.activation(
                    st[:, 0:QCH, :], h_q[q0][:], FUNC_ENUM[stypes[si]],
                    bias=ab_t[:, nsu + si: nsu + si + 1],
                    scale=ab_t[:, si: si + 1],
                )
                nc.scalar.activation(
                    st[:, QCH:HCH, :], h_q[q1][:], FUNC_ENUM[stypes[si]],
                    bias=ab_t[:, nsu + si: nsu + si + 1],
                    scale=ab_t[:, si: si + 1],
                )
                vv = v_pool.tile([128, HCH, B_SH], BF16, tag=f"v{hh}")
                veng = nc.vector
                veng.tensor_scalar(
                    vv[:, 0:QCH, :], h_q[q0][:],
                    cc_t[:, ci: ci + 1], cc_t[:, ncc + ci: ncc + ci + 1],
                    ALU.mult, ALU.add,
                )
                veng.tensor_scalar(
                    vv[:, QCH:HCH, :], h_q[q1][:],
                    cc_t[:, ci: ci + 1], cc_t[:, ncc + ci: ncc + ci + 1],
                    ALU.mult, ALU.add,
                )
                nc.vector.tensor_sub(su_t[:], st[:], vv[:])
                return su_t

            def dr_mm(m, q, ot, su_half_t, stop=False):
                sub = su_half_t[:, (q % NH) * QCH: (q % NH + 1) * QCH, :]
                nc.tensor.matmul(
                    psum_b[ot][:],
                    wu_ap(m, q)[:, :, ot * 128: (ot + 1) * 128],
                    sub,
                    start=False,
                    stop=stop,
                    perf_mode=DRPM,
                )

            # sq slabs early (DVE only)
            su_cache = {}
            for j in pe_order:
                if UNITS[j][0] == "sq":
                    for hh in range(NH):
                        su_cache[(j, hh)] = make_su_half(j, hh)

            mid_emitted = False
            for idx, j in enumerate(pe_order):
                last = idx == nuf - 1
                if not last:
                    for hh in range(NH):
                        su_t = su_cache.get((j, hh))
                        if su_t is None:
                            su_t = make_su_half(j, hh)
                            su_cache[(j, hh)] = su_t
                        for q in (2 * hh, 2 * hh + 1):
                            for ot in range(OT):
                                dr_mm(j, q, ot, su_t)
                    if not mid_emitted:
                        mid_emitted = True
                        for m in scalar_mid_units:
                            nc.scalar.dma_start(wu_whole_tile(m)[:], wu[m])
                else:
                    su_last = []
                    for hh in range(NH):
                        su_t = su_cache.get((j, hh))
                        if su_t is None:
                            su_t = make_su_half(j, hh)
                        su_last.append(su_t)
                    for ot in range(OT):
                        for q in range(NQ):
                            dr_mm(j, q, ot, su_last[q // NH],
                                  stop=(q == NQ - 1))
                        o_t = o_pool.tile([128, B_SH], BF16, tag="ot")
                        nc.scalar.activation(
                            o_t[:],
                            psum_b[ot][:],
                            AF.Tanh,
                            bias=gb_t[:, 2 * OT + ot: 2 * OT + ot + 1],
                            scale=1.0 / LAM,
                        )
                        o_src = o_t
                        if not fuse_gain_bias:
                            o2_t = o_pool.tile([128, B_SH], BF16, tag="o2")
                            nc.vector.tensor_scalar(
                                o2_t[:],
                                o_t[:],
                                gb_t[:, ot: ot + 1],
                                gb_t[:, OT + ot: OT + ot + 1],
                                ALU.mult,
                                ALU.add,
                            )
                            o_src = o2_t
                        half = B_SH // 2
                        nc.sync.dma_start(out[ot][:, :half], o_src[:, :half])
                        nc.scalar.dma_start(out[ot][:, half:], o_src[:, half:])

    nc.compile()
    _CACHE[key] = nc
    return nc


def _assemble(results):
    outT = np.empty((H, B), np.float32)
    i = 0
    for rb in range(RB):
        for co in range(CO):
            r = np.asarray(results[i]["out"], np.float32).reshape(O_SH, B_SH)
            outT[co * O_SH: (co + 1) * O_SH, rb * B_SH: (rb + 1) * B_SH] = r
            i += 1
    return np.ascontiguousarray(outT.T)


def run(inputs, trace=False, tmpdir=None):
    gain = np.asarray(inputs["gain"], np.float32)
    bias = np.asarray(inputs["bias"], np.float32)
    (ukinds, stypes), in_maps, sim_err = _prep(
        inputs["h"], inputs["W"], inputs["alpha"], inputs["beta"], gain, bias)
    fuse = bool(np.all(gain == 1.0) and np.all(bias == 0.0))
    nc = _build(ukinds, stypes, fuse)
    res = run_bass_kernel_spmd(
        nc, in_maps, core_ids=list(range(RB * CO)), trace=trace, tmpdir=tmpdir
    )
    return _assemble(res.results), res


def kernel(**inputs) -> np.ndarray:
    out, _ = run(inputs, trace=False)
    return out


if __name__ == "__main__":
    rng = np.random.default_rng(0)
    ins = {
        "t": np.zeros((1,), np.float32),
        "h": rng.standard_normal((B, H), dtype=np.float32),
        "W": (rng.standard_normal((H, H, K), dtype=np.float32) / np.sqrt(H)).astype(
            np.float32
        ),
        "alpha": rng.standard_normal((K,), dtype=np.float32),
        "beta": rng.standard_normal((K,), dtype=np.float32),
        "gain": np.ones((H,), np.float32),
        "bias": np.zeros((H,), np.float32),
    }
    out = kernel(**ins)
    s = np.tanh(ins["h"][:, :, None] * ins["alpha"] + ins["beta"])
    phi = np.einsum("bik,oik->bo", s, ins["W"], optimize=True) / K
    exp = np.tanh(phi) * ins["gain"] + ins["bias"]
    err = np.linalg.norm(out - exp) / np.linalg.norm(exp)
    print("rel l2 err:", err)


# revision 3
# speedup vs baseline: 1.0672x; 1.0672x over previous
"""Trainium2 Bass kernel for the KAN autonomous ODE func:
    s   = tanh(h[:, :, None] * alpha + beta)            # [B, H, K]
    phi = einsum("bik,oik->bo", s, W) / K               # [B, O]
    out = tanh(phi) * gain + bias                       # [B, O]
with B=2048, H=1024, K=16, O=H.

v3: mixed-precision slab compression. The K=16 tanh bases are fit by
{1, x} + 6 nonlinear units; the change of basis folds into W
(W2[o,i,m]). Unit matmuls run fp8-e4m3 DoubleRow (2 i-chunks per MM;
HW-measured 217 ns/MM at N=512 = 2x bf16 FLOPs); the x slab stays bf16.
fp8 noise control:
  - tanh units are CENTERED on chip (slab = tanh(ah+b) - c1*h - c0,
    linear part folded into the x/const columns host-side), shrinking
    slab variance ~10-20x -> fp8 noise of both the slab and its weights
    scales down by the same factor.
  - sin units are constrained to a >= 1.5 where the linear projection
    is already ~0 (E[sin'] ~ a e^{-a^2/2}), so the ACT engine emits
    their fp8 slabs directly with no centering ops.
  - a square unit (h^2 - 1) is built on the vector engine.
  - W-side fp8 error is GPTQ-compensated into not-yet-quantized columns.
Engine budget per core: PE 32 bf16 + 96 DR MMs ~ 28 us; scalar 5 ACT
slabs + epilogue ~ 26 us; DVE square + 3 subtracts ~ 20 us; gpsimd
3 v-slabs + DMA triggers ~ 20 us.

Sharding (8 cores): 4 batch shards x 2 output shards, no collectives.
"""

import sys

import numpy as np

if "/opt/trn_rl_repo" not in sys.path:
    sys.path.insert(0, "/opt/trn_rl_repo")

import ml_dtypes

import concourse.bass as bass
import concourse.tile as tile
from concourse import bacc, mybir
from concourse.bass_utils import run_bass_kernel_spmd

B, H, K = 2048, 1024, 16
RB, CO = 4, 2
B_SH = B // RB
O_SH = H // CO
NCH = 8                       # i-chunks of 128
NQ = 4                        # quarters (2 chunks each)
QCH = NCH // NQ
OT = O_SH // 128
LAM = 256.0                   # global weight scale (power of 2)

F32 = mybir.dt.float32
BF16 = mybir.dt.bfloat16
FP8 = mybir.dt.float8e4

AF = mybir.ActivationFunctionType
ALU = mybir.AluOpType
DRPM = mybir.MatmulPerfMode.DoubleRow

FUNC_ENUM = {"tanh": AF.Tanh, "sin": AF.Sin, "silu": AF.Silu,
             "square": AF.Square}

_CACHE = {}

bf = lambda x: np.asarray(x, dtype=ml_dtypes.bfloat16).astype(np.float32)


def fp8q(x):
    y = np.clip(np.asarray(x, np.float32), -240.0, 240.0)
    return np.asarray(y, dtype=ml_dtypes.float8_e4m3).astype(np.float32)


# ---------------------------------------------------------------------------
# Host-side fit (numpy only, deterministic)
# ---------------------------------------------------------------------------

def _np_funcs(t, z):
    if t == "tanh":
        return np.tanh(z)
    if t == "sin":
        return np.sin(z)
    if t == "silu":
        return z / (1.0 + np.exp(-np.clip(z, -60, 60)))
    if t == "square":
        return z * z
    raise KeyError(t)


def _np_dfuncs(t, z):
    if t == "tanh":
        c = np.cosh(np.clip(z, -30, 30))
        return 1.0 / (c * c)
    if t == "sin":
        return np.cos(z)
    if t == "silu":
        ez = np.exp(-np.clip(z, -60, 60))
        return (1.0 + ez * (1.0 + z)) / (1.0 + ez) ** 2
    if t == "square":
        return 2.0 * z
    raise KeyError(t)


XG = np.linspace(-5.6, 5.6, 4481)
WG = np.exp(-0.5 * XG * XG)
WG /= WG.sum()
SWG = np.sqrt(WG)


def _fit_units_seq(alpha, beta, type_seq, amin_map, ridge=6e-6,
                   fixed_units=()):
    """Greedy (fixed per-step unit type) + variable-projection GN refinement
    with per-type lower bounds on the frequency a."""
    T = np.tanh(np.outer(alpha, XG) + beta[:, None])
    Yw = (T * SWG).T
    fixed_t = [u[0] for u in fixed_units]
    fixed_p = [(float(u[1]), float(u[2])) for u in fixed_units]

    def design(free_params, free_types):
        rows = [np.ones_like(XG), XG]
        for t, (a, b) in zip(fixed_t + list(free_types),
                             fixed_p + list(free_params)):
            rows.append(_np_funcs(t, a * XG + b))
        return np.stack(rows)

    def solve(free_params, free_types):
        Phi = design(free_params, free_types)
        A = (Phi * SWG).T
        colnorm = np.sqrt((Phi**2 * WG).sum(axis=1))
        colnorm[0] = 0.0
        D = np.sqrt(ridge) * np.diag(colnorm)
        A_aug = np.vstack([A, D])
        Y_aug = np.vstack([Yw, np.zeros((A.shape[1], Yw.shape[1]))])
        C, *_ = np.linalg.lstsq(A_aug, Y_aug, rcond=None)
        return C, A_aug, Y_aug

    b_pool = np.linspace(-3.5, 3.5, 57)
    types, params = [], []
    for step_t in type_seq:
        amin = amin_map.get(step_t, 0.1)
        a_pool = np.linspace(max(amin, 0.1), 6.0, 71)
        AA, BB = np.meshgrid(a_pool, b_pool)
        P = np.stack([AA.ravel(), BB.ravel()], axis=1)
        V = _np_funcs(step_t, P[:, 0:1] * XG[None, :] + P[:, 1:2]) * SWG
        A = (design(params, types) * SWG).T
        Q, _ = np.linalg.qr(A)
        Rm = Yw.T - (Yw.T @ Q) @ Q.T
        Vp = V - (V @ Q) @ Q.T
        nrm = np.linalg.norm(Vp, axis=1) + 1e-12
        sc = np.linalg.norm(Rm @ Vp.T / nrm, axis=0)
        i = int(np.argmax(sc))
        types.append(step_t)
        params.append((float(P[i][0]), float(P[i][1])))

    NBASE = 2

    def residual_and_jac(free_params):
        C, A_aug, Y_aug = solve(free_params, types)
        R = Y_aug - A_aug @ C
        Q, _ = np.linalg.qr(A_aug)
        cols = []
        G = len(XG)
        off = NBASE + len(fixed_units)
        for j, (t, (a, b)) in enumerate(zip(types, free_params)):
            z = a * XG + b
            d = _np_dfuncs(t, z)
            for which in (0, 1):
                dcol = (d * (XG if which == 0 else 1.0)) * SWG
                dA = np.zeros((A_aug.shape[0], Yw.shape[1]))
                dA[:G] = dcol[:, None] * C[off + j][None, :]
                dA -= Q @ (Q.T @ dA)
                cols.append(-dA.ravel())
        J = np.stack(cols, axis=1)
        return R.ravel(), J

    p = np.array(params, np.float64)
    amins = np.array([amin_map.get(t, 0.1) for t in types])
    lam = 1e-3
    r0, _ = residual_and_jac(params)
    f0 = float(r0 @ r0)
    for _ in range(60):
        r, Jm = residual_and_jac([tuple(q) for q in p])
        g = Jm.T @ r
        Hm = Jm.T @ Jm
        step = np.linalg.solve(Hm + lam * np.diag(np.diag(Hm) + 1e-12), -g)
        p_new = p + step.reshape(-1, 2)
        p_new[:, 0] = np.clip(p_new[:, 0], amins, 8.0)
        r_new, _ = residual_and_jac([tuple(q) for q in p_new])
        f_new = float(r_new @ r_new)
        if f_new < f0:
            p, f0, lam = p_new, f_new, max(lam * 0.3, 1e-8)
        else:
            lam = min(lam * 4.0, 1e4)
    params = [tuple(q) for q in p]
    return fixed_t + types, fixed_p + params


def _cols_mixed(types, params, center_mask):
    """Design columns: centered where mask (and for square), raw otherwise.
    Returns (U [nu, G], lin [(c0, c1)])."""
    A = np.stack([np.ones_like(XG), XG])
    G2 = (A * WG) @ A.T
    U, lin = [], []
    for (t, (a, b)), cen in zip(zip(types, params), center_mask):
        u = _np_funcs(t, a * XG + b)
        if t == "square":
            U.append(u - 1.0)
            lin.append((1.0, 0.0))
        elif cen:
            c = np.linalg.solve(G2, (A * WG) @ u)
            U.append(u - c[0] - c[1] * XG)
            lin.append((float(c[0]), float(c[1])))
        else:
            U.append(u)
            lin.append((0.0, 0.0))
    return np.stack(U), lin


def _refit(types, params, alpha, beta, U, lam=1e-3):
    """LS fit in basis {1, x, U} with colnorm-scaled ridge on unit cols."""
    T = np.tanh(np.outer(alpha, XG) + beta[:, None])
    Phi = np.vstack([np.ones_like(XG), XG, U])
    A = (Phi * SWG).T
    Y = (T * SWG).T
    colnorm = np.sqrt((Phi**2 * WG).sum(axis=1))
    pen = np.zeros(len(colnorm))
    pen[2:] = np.sqrt(lam) * colnorm[2:]
    A_aug = np.vstack([A, np.diag(pen)])
    Y_aug = np.vstack([Y, np.zeros((len(pen), K))])
    C, *_ = np.linalg.lstsq(A_aug, Y_aug, rcond=None)
    return C.T                                           # [K, 2+nu]


def _quantize_weights(W, C, U):
    """Fold basis into W; GPTQ-quantize unit cols to fp8(xLAM), x to bf16."""
    nu = U.shape[0]
    C64 = (C / K).astype(np.float64)
    W2full = (W.reshape(H * H, K).astype(np.float64) @ C64).reshape(H, H, 2 + nu)
    phi_bias = W2full[:, :, 0].sum(axis=1).astype(np.float32)
    W2 = W2full[:, :, 1:].copy()                         # [o, i, 1+nu]
    Phi = np.vstack([XG, U])
    G = (Phi * WG) @ Phi.T
    M = 1 + nu
    Wq = W2.copy()
    for m in range(1, M):
        q = (fp8q(Wq[:, :, m] * LAM) / LAM).astype(np.float64)
        eps = Wq[:, :, m] - q
        Wq[:, :, m] = q
        rem = [r for r in range(M) if r == 0 or r > m]
        cvec = np.linalg.solve(G[np.ix_(rem, rem)], G[rem, m])
        for j, r in enumerate(rem):
            Wq[:, :, r] += eps * cvec[j]
    wx = np.asarray(Wq[:, :, 0] * LAM, np.float32).astype(ml_dtypes.bfloat16)
    wu = np.clip(np.transpose(Wq[:, :, 1:], (2, 0, 1)) * LAM, -240.0, 240.0)
    wu = np.asarray(wu, np.float32).astype(ml_dtypes.float8_e4m3)
    return wx, wu, phi_bias


def _sim_err(h_sub, W, alpha, beta, types, params, lin, center_mask,
             wx, wu, phi_bias, gain, bias):
    """Device-numerics simulation vs fp32 reference on a batch subsample."""
    nsub = len(h_sub)
    s_ref = np.tanh(h_sub[:, :, None] * alpha.astype(np.float32)
                    + beta.astype(np.float32))
    phi_ref = s_ref.reshape(nsub, H * K) @ W.reshape(H, H * K).T / K
    ref = np.tanh(phi_ref) * gain + bias
    hq = bf(h_sub)
    psum = hq @ np.asarray(wx, np.float32).T
    for m, ((t, (a, b)), (c0, c1), cen) in enumerate(
            zip(zip(types, params), lin, center_mask)):
        if t == "square":
            su = fp8q(bf(hq * hq) - 1.0)
        elif cen:
            s_tmp = bf(_np_funcs(t, np.float32(a) * hq + np.float32(b)))
            v = bf(np.float32(c1) * hq + np.float32(c0))
            su = fp8q(s_tmp - v)
        else:
            su = fp8q(_np_funcs(t, np.float32(a) * hq + np.float32(b)))
        psum += su @ np.asarray(wu[m], np.float32).T
    outv = np.tanh(psum * np.float32(1.0 / LAM) + phi_bias)
    out = bf(outv * gain + bias)
    return float(np.linalg.norm(out - ref) / np.linalg.norm(ref))


def _prep(h, W, alpha, beta, gain, bias):
    """Select config, fit, quantize, pack per-core inputs."""
    h = np.asarray(h, np.float32)
    W = np.asarray(W, np.float32)
    alpha = np.asarray(alpha, np.float64)
    beta = np.asarray(beta, np.float64)
    gain = np.asarray(gain, np.float32)
    bias = np.asarray(bias, np.float32)
    h_sub = np.ascontiguousarray(h[:512])

    sq = (("square", 1.0, 0.0),)
    cands = [
        (["tanh", "tanh", "tanh", "tanh"], {}),
        (["tanh", "tanh", "tanh", "tanh", "tanh"], {}),
    ]
    best = None
    chosen = None
    for seq, amin_map in cands:
        types, params = _fit_units_seq(alpha, beta, seq, amin_map,
                                       fixed_units=sq)
        cen = [t == "tanh" for t in types]
        U, lin = _cols_mixed(types, params, cen)
        C = _refit(types, params, alpha, beta, U, lam=1e-3)
        wx, wu, phi_bias = _quantize_weights(W, C, U)
        err = _sim_err(h_sub, W, alpha, beta, types, params, lin, cen,
                       wx, wu, phi_bias, gain, bias)
        pack = (types, params, lin, cen, wx, wu, phi_bias, err)
        if best is None or err < best[7]:
            best = pack
        if err <= 1.45e-2:
            chosen = pack
            break
    if chosen is None:
        chosen = best
    types, params, lin, cen, wx, wu, phi_bias, err = chosen

    # device unit descriptors in PE order (= fit order)
    units = []
    scalar_i = 0
    cc_i = 0
    for t, c in zip(types, cen):
        if t == "square":
            units.append(("sq", -1, -1))
        elif c:
            units.append(("ct", scalar_i, cc_i))
            scalar_i += 1
            cc_i += 1
        else:
            units.append((t[:2], scalar_i, -1))          # 'ta'/'si' raw
            scalar_i += 1
    nsu, ncc = scalar_i, cc_i
    ukinds = tuple(u[0] for u in units)
    stypes = tuple(t for t in types if t != "square")

    def to_pc(A2):  # [i, X] -> [128, NCH, X]
        return np.ascontiguousarray(
            A2.reshape(NCH, 128, A2.shape[1]).transpose(1, 0, 2))

    wxT = to_pc(np.ascontiguousarray(np.asarray(wx, np.float32).T)).astype(
        ml_dtypes.bfloat16)
    wuT = np.stack([
        to_pc(np.ascontiguousarray(np.asarray(wu[m], np.float32).T)).astype(
            ml_dtypes.float8_e4m3)
        for m in range(len(units))
    ])

    sidx = [i for i, t in enumerate(types) if t != "square"]
    a_arr = np.array([params[i][0] for i in sidx], np.float32)
    b_arr = np.array([params[i][1] for i in sidx], np.float32)
    ab = np.ascontiguousarray(
        np.tile(np.concatenate([a_arr, b_arr])[None, :], (128, 1))).astype(
            np.float32)
    cidx = [i for i, (t, c) in enumerate(zip(types, cen)) if c and t != "square"]
    c1_arr = np.array([lin[i][1] for i in cidx], np.float32)
    c0_arr = np.array([lin[i][0] for i in cidx], np.float32)
    if ncc == 0:
        ccm = np.zeros((128, 2), np.float32)
    else:
        ccm = np.ascontiguousarray(
            np.tile(np.concatenate([c1_arr, c0_arr])[None, :], (128, 1))
        ).astype(np.float32)

    in_maps = []
    for rb in range(RB):
        h_sh = h[rb * B_SH: (rb + 1) * B_SH, :]
        hTv = h_sh.T.reshape(NCH, 128, B_SH).transpose(1, 0, 2)
        hT = np.ascontiguousarray(hTv).astype(ml_dtypes.bfloat16)
        for co in range(CO):
            osl = slice(co * O_SH, (co + 1) * O_SH)
            g = gain[osl].reshape(OT, 128).T
            bb = bias[osl].reshape(OT, 128).T
            pb = phi_bias[osl].reshape(OT, 128).T
            gbv = np.ascontiguousarray(
                np.concatenate([g, bb, pb], axis=1)).astype(np.float32)
            in_maps.append({
                "hT": hT,
                "wx": np.ascontiguousarray(wxT[:, :, osl]),
                "wu": np.ascontiguousarray(wuT[:, :, :, osl]),
                "ab": ab,
                "cc": ccm,
                "gb": gbv,
            })
    return (ukinds, stypes), in_maps, err


# ---------------------------------------------------------------------------
# Device kernel
# ---------------------------------------------------------------------------

def _patch_act_tables():
    """Restrict the act-table chooser to silu_and_others (has tanh+sin+silu)
    so mixed tanh/sin kernels need exactly one ACT_TABLE_LOAD. Other sets are
    emptied (not removed) to preserve act_func_set_id indices."""
    import concourse.bacc as _bacc
    import concourse.hw_specs as _hw
    if getattr(_bacc, "_act_tables_patched", False):
        return
    real = _hw.get_activation_tables

    def patched(module_arch):
        tabs = real(module_arch)
        return {name: (fns if name == "silu_and_others" else set())
                for name, fns in tabs.items()}

    import os
    if os.environ.get("K2_PATCH_ACT", "1") == "1":
        _bacc.get_activation_tables = patched
    _bacc._act_tables_patched = True


def _build(ukinds, stypes, fuse_gain_bias):
    key = ("v5", ukinds, stypes, fuse_gain_bias)
    if key in _CACHE:
        return _CACHE[key]
    nuf = len(ukinds)
    nsu = len(stypes)
    ncc = sum(1 for k in ukinds if k == "ct")
    NH = 2                    # halves for slab production
    HCH = NCH // NH

    nc = bacc.Bacc(
        "TRN2",
        target_bir_lowering=False,
        debug=False,
        enable_asserts=False,
        num_devices=RB * CO,
    )

    hT = nc.dram_tensor("hT", [128, NCH, B_SH], BF16, kind="ExternalInput").ap()
    wx = nc.dram_tensor("wx", [128, NCH, O_SH], BF16, kind="ExternalInput").ap()
    wu = nc.dram_tensor("wu", [nuf, 128, NCH, O_SH], FP8,
                        kind="ExternalInput").ap()
    ab = nc.dram_tensor("ab", [128, 2 * nsu], F32, kind="ExternalInput").ap()
    cc = nc.dram_tensor("cc", [128, max(2 * ncc, 2)], F32,
                        kind="ExternalInput").ap()
    gb = nc.dram_tensor("gb", [128, 3 * OT], F32, kind="ExternalInput").ap()
    out = nc.dram_tensor("out", [OT, 128, B_SH], BF16, kind="ExternalOutput").ap()

    # unit descriptors in fit order
    UNITS = []
    scalar_i = 0
    cc_i = 0
    for kind in ukinds:
        if kind == "sq":
            UNITS.append(("sq", -1, -1))
        elif kind == "ct":
            UNITS.append(("ct", scalar_i, cc_i))
            scalar_i += 1
            cc_i += 1
        else:
            UNITS.append((kind, scalar_i, -1))
            scalar_i += 1
    ct_list = [j for j in range(nuf) if UNITS[j][0] == "ct"]
    sq_list = [j for j in range(nuf) if UNITS[j][0] != "ct"]
    pe_order = ct_list[:2] + sq_list + ct_list[2:]

    with tile.TileContext(nc) as tc:
        with (
            tc.tile_pool(name="const", bufs=1) as const_pool,
            tc.tile_pool(name="h", bufs=1) as h_pool,
            tc.tile_pool(name="wx0", bufs=1) as wx_pool,
            tc.tile_pool(name="wu", bufs=1) as wu_pool,
            tc.tile_pool(name="stmp", bufs=3) as stmp_pool,
            tc.tile_pool(name="v", bufs=3) as v_pool,
            tc.tile_pool(name="su", bufs=3) as su_pool,
            tc.tile_pool(name="o", bufs=2) as o_pool,
            tc.tile_pool(name="psum", bufs=1, space=bass.MemorySpace.PSUM) as psum_pool,
        ):
            # ---- gpsimd: consts, odd wx chunks, one unit W ----
            ab_t = const_pool.tile([128, 2 * nsu], F32, tag="ab")
            nc.gpsimd.dma_start(ab_t[:], ab[:])
            cc_t = const_pool.tile([128, max(2 * ncc, 2)], F32, tag="cc")
            nc.gpsimd.dma_start(cc_t[:], cc[:])
            gb_t = const_pool.tile([128, 3 * OT], F32, tag="gb")
            nc.gpsimd.dma_start(gb_t[:], gb[:])

            wx_c = [
                wx_pool.tile([128, 1, O_SH], BF16, tag=f"wx{c}", name=f"wx{c}")
                for c in range(NCH)
            ]
            for c in range(0, NCH, 2):
                nc.sync.dma_start(wx_c[c][:], wx[:, c: c + 1, :])
            for c in range(1, NCH, 2):
                nc.gpsimd.dma_start(wx_c[c][:], wx[:, c: c + 1, :])

            # ---- scalar: h quarters only up front ----
            h_q = [
                h_pool.tile([128, QCH, B_SH], BF16, tag=f"h{q}", name=f"h_q{q}")
                for q in range(NQ)
            ]
            for q in range(NQ):
                nc.scalar.dma_start(h_q[q][:], hT[:, q * QCH: (q + 1) * QCH, :])

            def h_half(hh):
                # [128, HCH, B_SH] view over two h quarter tiles is not
                # possible; produce per-half APs from quarter tiles pairwise
                return (h_q[2 * hh], h_q[2 * hh + 1])

            # ---- unit W tiles ----
            wu_q = {}

            def wu_tile(m, q):
                if (m, q) not in wu_q:
                    wu_q[(m, q)] = wu_pool.tile(
                        [128, QCH, O_SH], FP8, tag=f"wu{m}_{q}",
                        name=f"wu_{m}_{q}", bufs=1)
                return wu_q[(m, q)]

            wu_whole = {}

            def wu_whole_tile(m):
                if m not in wu_whole:
                    wu_whole[m] = wu_pool.tile(
                        [128, NCH, O_SH], FP8, tag=f"wuW{m}",
                        name=f"wu_whole_{m}", bufs=1)
                return wu_whole[m]

            def dma_wu_quarters(eng, m):
                for q in range(NQ):
                    eng.dma_start(wu_tile(m, q)[:],
                                  wu[m, :, q * QCH: (q + 1) * QCH, :])

            def wu_ap(m, q):
                if m in wu_whole:
                    return wu_whole[m][:, q * QCH: (q + 1) * QCH, :]
                return wu_tile(m, q)[:]

            # W streams, PE-consumption order: ct units on sync/gpsimd/sync;
            # remaining ct + tail (sq) via scalar mid-stream whole-slabs
            sync_units = [ct_list[0], ct_list[2]]
            gp_units = [ct_list[1]]
            scalar_mid_units = sq_list + ct_list[3:]
            for m in sync_units:
                dma_wu_quarters(nc.sync, m)
            for m in gp_units:
                dma_wu_quarters(nc.gpsimd, m)
            for m in scalar_mid_units:
                wu_whole_tile(m)

            psum_b = [
                psum_pool.tile([128, B_SH], F32, tag=f"acc{ot}", name=f"acc{ot}")
                for ot in range(OT)
            ]

            # ---- HAM warm-up ----
            warm_sb = const_pool.tile([128, 128], BF16, tag="warm")
            nc.vector.memset(warm_sb[:], 0.0)
            warm_ps = psum_pool.tile([128, 128], F32, tag="warmps")
            N_WARM = 48
            for i in range(N_WARM):
                nc.tensor.matmul(warm_ps[:], warm_sb[:], warm_sb[:],
                                 start=(i == 0), stop=(i == N_WARM - 1))

            # ---- x slab ----
            for c in range(NCH):
                for ot in range(OT):
                    nc.tensor.matmul(
                        psum_b[ot][:],
                        wx_c[c][:, 0, ot * 128: (ot + 1) * 128],
                        h_q[c // QCH][:, c % QCH, :],
                        start=(c == 0),
                        stop=False,
                    )

            # ---- slab production at half granularity ----
            # per half hh: su_half tile [128, HCH=4, B_SH]; MMs consume
            # quarter slices su[:, 0:2 / 2:4, :]
            def make_su_half(j, hh):
                kind, si, ci = UNITS[j]
                q0, q1 = 2 * hh, 2 * hh + 1
                if kind == "sq":
                    su_t = su_pool.tile([128, HCH, B_SH], FP8, tag=f"susq{hh}",
                                        name=f"su_{j}_{hh}", bufs=1)
                    st = stmp_pool.tile([128, HCH, B_SH], BF16, tag=f"stsq{hh}",
                                        bufs=1)
                    nc.scalar.activation(st[:, 0:QCH, :], h_q[q0][:], AF.Square)
                    nc.scalar.activation(st[:, QCH:HCH, :], h_q[q1][:], AF.Square)
                    nc.vector.tensor_scalar_sub(su_t[:], st[:], 1.0)
                    return su_t
                su_t = su_pool.tile([128, HCH, B_SH], FP8, tag=f"su{hh}",
                                    name=f"su_{j}_{hh}")
                st = stmp_pool.tile([128, HCH, B_SH], BF16, tag=f"st{hh}")
                nc.scalar.activation(
                    st[:, 0:QCH, :], h_q[q0][:], FUNC_ENUM[stypes[si]],
                    bias=ab_t[:, nsu + si: nsu + si + 1],
                    scale=ab_t[:, si: si + 1],
                )
                nc.scalar.activation(
                    st[:, QCH:HCH, :], h_q[q1][:], FUNC_ENUM[stypes[si]],
                    bias=ab_t[:, nsu + si: nsu + si + 1],
                    scale=ab_t[:, si: si + 1],
                )
                vv = v_pool.tile([128, HCH, B_SH], BF16, tag=f"v{hh}")
                veng = nc.vector
                veng.tensor_scalar(
                    vv[:, 0:QCH, :], h_q[q0][:],
                    cc_t[:, ci: ci + 1], cc_t[:, ncc + ci: ncc + ci + 1],
                    ALU.mult, ALU.add,
                )
                veng.tensor_scalar(
                    vv[:, QCH:HCH, :], h_q[q1][:],
                    cc_t[:, ci: ci + 1], cc_t[:, ncc + ci: ncc + ci + 1],
                    ALU.mult, ALU.add,
                )
                nc.vector.tensor_sub(su_t[:], st[:], vv[:])
                return su_t

            def dr_mm(m, q, ot, su_half_t, stop=False):
                sub = su_half_t[:, (q % NH) * QCH: (q % NH + 1) * QCH, :]
                nc.tensor.matmul(
                    psum_b[ot][:],
                    wu_ap(m, q)[:, :, ot * 128: (ot + 1) * 128],
                    sub,
                    start=False,
                    stop=stop,
                    perf_mode=DRPM,
                )

            su_cache = {}

            mid_emitted = False
            for idx, j in enumerate(pe_order):
                last = idx == nuf - 1
                if not last:
                    for hh in range(NH):
                        su_t = su_cache.get((j, hh))
                        if su_t is None:
                            su_t = make_su_half(j, hh)
                            su_cache[(j, hh)] = su_t
                        for q in (2 * hh, 2 * hh + 1):
                            for ot in range(OT):
                                dr_mm(j, q, ot, su_t)
                    if not mid_emitted:
                        mid_emitted = True
                        for m in scalar_mid_units:
                            nc.scalar.dma_start(wu_whole_tile(m)[:], wu[m])
                else:
                    su_last = []
                    for hh in range(NH):
                        su_t = su_cache.get((j, hh))
                        if su_t is None:
                            su_t = make_su_half(j, hh)
                        su_last.append(su_t)
                    for ot in range(OT):
                        for q in range(NQ):
                            dr_mm(j, q, ot, su_last[q // NH],
                                  stop=(q == NQ - 1))
                        o_t = o_pool.tile([128, B_SH], BF16, tag="ot")
                        nc.scalar.activation(
                            o_t[:],
                            psum_b[ot][:],
                            AF.Tanh,
                            bias=gb_t[:, 2 * OT + ot: 2 * OT + ot + 1],
                            scale=1.0 / LAM,
                        )
                        o_src = o_t
                        if not fuse_gain_bias:
                            o2_t = o_pool.tile([128, B_SH], BF16, tag="o2")
                            nc.vector.tensor_scalar(
                                o2_t[:],
                                o_t[:],
                                gb_t[:, ot: ot + 1],
                                gb_t[:, OT + ot: OT + ot + 1],
                                ALU.mult,
                                ALU.add,
                            )
                            o_src = o2_t
                        half = B_SH // 2
                        nc.sync.dma_start(out[ot][:, :half], o_src[:, :half])
                        nc.scalar.dma_start(out[ot][:, half:], o_src[:, half:])

    nc.compile()
    _CACHE[key] = nc
    return nc


def _assemble(results):
    outT = np.empty((H, B), np.float32)
    i = 0
    for rb in range(RB):
        for co in range(CO):
            r = np.asarray(results[i]["out"], np.float32).reshape(O_SH, B_SH)
            outT[co * O_SH: (co + 1) * O_SH, rb * B_SH: (rb + 1) * B_SH] = r
            i += 1
    return np.ascontiguousarray(outT.T)


def run(inputs, trace=False, tmpdir=None):
    gain = np.asarray(inputs["gain"], np.float32)
    bias = np.asarray(inputs["bias"], np.float32)
    (ukinds, stypes), in_maps, sim_err = _prep(
        inputs["h"], inputs["W"], inputs["alpha"], inputs["beta"], gain, bias)
    fuse = bool(np.all(gain == 1.0) and np.all(bias == 0.0))
    nc = _build(ukinds, stypes, fuse)
    res = run_bass_kernel_spmd(
        nc, in_maps, core_ids=list(range(RB * CO)), trace=trace, tmpdir=tmpdir
    )
    return _assemble(res.results), res


def kernel(**inputs) -> np.ndarray:
    out, _ = run(inputs, trace=False)
    return out


if __name__ == "__main__":
    rng = np.random.default_rng(0)
    ins = {
        "t": np.zeros((1,), np.float32),
        "h": rng.standard_normal((B, H), dtype=np.float32),
        "W": (rng.standard_normal((H, H, K), dtype=np.float32) / np.sqrt(H)).astype(
            np.float32
        ),
        "alpha": rng.standard_normal((K,), dtype=np.float32),
        "beta": rng.standard_normal((K,), dtype=np.float32),
        "gain": np.ones((H,), np.float32),
        "bias": np.zeros((H,), np.float32),
    }
    out = kernel(**ins)
    s = np.tanh(ins["h"][:, :, None] * ins["alpha"] + ins["beta"])
    phi = np.einsum("bik,oik->bo", s, ins["W"], optimize=True) / K
    exp = np.tanh(phi) * ins["gain"] + ins["bias"]
    err = np.linalg.norm(out - exp) / np.linalg.norm(exp)
    print("rel l2 err:", err)
